# revision 87
# baseline (speedup 1.0000x reference)
"""Causal attention + output projection on 8 Trainium2 NeuronCores.

Problem (hardcoded): B=2, H=12, T=2048, D=64, DIM=768, fp32 in/out.

Sharding: 24 (b, h) pairs -> 3 heads per core; cores 0-3 take b=0,
cores 4-7 take b=1.  Each core computes attention for its 3 heads plus
the partial output projection sum_h y_h @ W[h*64:(h+1)*64, :] as a
(T, DIM) bf16 partial; the host sums the 4 partials per batch in f32.
No collectives.

Device-side layout is fully transposed ([s, q]); all matmul operands
are bf16 (1 PE cycle/row vs 4 for fp32):
  - qkt: [64, 2T] per head = qT/sqrt(D) | kT  (no padding rows)
  - vat: [128, T] per head: 16 blocks of [v-tile | ones] so one PV
    matmul yields y^T (rows 0:64) and softmax denominators (64:128)
  - ebias: exp(bias^T) with exact 0 on causally-masked positions,
    stored trimmed: per (q-chunk j, s-tile i) only cols >= c0 where
    c0 = max(0, 128 i - 512 j).  No bias matmuls: QK writes PSUM with
    start=True and DVE multiplies exp(logits) * ebias (2x bf16 mode).

Loop structure: q-chunk OUTER, head INNER, so the projection of chunk
j-1 (which needs all heads) interleaves between chunk j's heads and
the PE/ACT/DVE load stays even across the whole timeline.  Per
(chunk j, head, group of G=2 s-tiles): QK matmuls (trimmed) -> exp
(trimmed per-bank on diagonal groups, packed into the pe tile) -> one
DVE mult per group -> PV matmuls (trimmed) into psy; then
reciprocal+normalize into yT.  Heads 0,1 pack one [128, T] yT tile
(h1 on partitions 64:128) so projection contracts 128 deep; head 2 is
[64, T].  Projection output stages through PSUM -> bf16 SBUF -> HBM;
the staging copies run on DVE mid-kernel and on ACT for the tail
blocks (ACT is idle once the last exp retires).

All DMAs issue from SP/gpsimd queues only: a DMA blocks its issuing
engine's sequencer for the whole transfer in this cost model, and
transfers serialize globally at ~360 GB/s, so ACT/PE/DVE stay clear
of DMA duty.  First-chunk operands are split into small leading DMAs
so the first QK starts ~1us in.

The tail projection borrows the 3-slot psl PSUM pool (drained by
then) for pipelining, alternates its staging copies between DVE and
ACT, and the final head's normalize is split in column halves so the
first tail blocks start early.

Engine busy (CoreSim, of 72.3us total): DVE 61.3 (mult/norm/recip/
copies), ACT 58.7 (exp + tail copies), PE 53.9 (matmuls), SP/Pool
(DMA queues); DVE runs ~97% occupied across its span.  The last tail
out-DMA issues from the SP (HWDGE) queue, whose fixed issue cost is
lower than SWDGE's, trimming the drain.
"""

import math

import numpy as np
import ml_dtypes

B, H, T, D = 2, 12, 2048, 64
DIM = H * D
NCORES = 8
HPC = 3           # heads per core
P = 128
QC = 512          # q-chunk width
NJ = T // QC      # 4 q-chunks
NT = T // P       # 16 s-tiles
G = 2             # s-tiles per PSUM logits group

# causal trim tables: chunk j, s-tile i -> start col c0, width w
_C0 = {}
_W = {}
_BOFF = {}        # (j, i) -> col offset of trimmed tile in ebias row
_CHUNK_OFF = {}   # j -> start col of chunk j's region
_acc = 0
for _j in range(NJ):
    _CHUNK_OFF[_j] = _acc
    for _i in range(4 * (_j + 1)):
        _c0 = max(0, P * _i - QC * _j)
        _C0[(_j, _i)] = _c0
        _W[(_j, _i)] = QC - _c0
        _BOFF[(_j, _i)] = _acc
        _acc += QC - _c0
SUMW = _acc       # 17408 trimmed bias cols per head
_CHUNK_OFF[NJ] = SUMW
WMAX = _CHUNK_OFF[NJ] - _CHUNK_OFF[NJ - 1]  # widest chunk region (7424)

_PROGRAM = None


def _build_program():
    import concourse.bass as bass
    import concourse.mybir as mybir
    import concourse.tile as tile
    from concourse import bacc
    from contextlib import ExitStack

    dt = mybir.dt
    f32 = dt.float32
    bf16 = dt.bfloat16
    EXP = mybir.ActivationFunctionType.Exp
    ds = bass.ds

    nc = bacc.Bacc("TRN2", num_devices=NCORES)
    vat = nc.declare_dram_parameter("vat", [HPC * P, T], bf16, isOutput=False)
    qkt = nc.declare_dram_parameter("qkt", [HPC * D, 2 * T], bf16, isOutput=False)
    ebias = nc.declare_dram_parameter("ebias", [HPC * P, SUMW], bf16, isOutput=False)
    wproj = nc.declare_dram_parameter("wproj", [P, 2 * DIM], bf16, isOutput=False)
    out = nc.declare_dram_parameter("out", [T, DIM], bf16, isOutput=True)

    with tile.TileContext(nc) as tc, ExitStack() as ctx:
        pers = ctx.enter_context(tc.tile_pool(name="pers", bufs=1))
        w2 = pers.tile([P, 2 * DIM], bf16)
        qk_t = [pers.tile([D, 2 * T], bf16, name=f"qk{h}") for h in range(HPC)]
        va_t = [pers.tile([P, T], bf16, name=f"va{h}") for h in range(HPC)]
        yT2 = pers.tile([P, T], bf16)   # heads 0,1 (h1 on partitions 64:)
        yTs = pers.tile([D, T], bf16)   # head 2
        o_big = pers.tile([P, NT * DIM], bf16)

        # prologue DMAs: critical first-chunk slices lead, rests follow.
        # SP carries heads 0,1; gpsimd carries head 2 + w.
        def eng(h):
            return nc.sync if h < 2 else nc.gpsimd

        for h in range(HPC):
            e = eng(h)
            e.dma_start(qk_t[h][:, 0:QC], qkt[ds(h * D, D), 0:QC])
            e.dma_start(qk_t[h][:, T : T + QC], qkt[ds(h * D, D), T : T + QC])

        with (
            tc.tile_pool(name="eb", bufs=3) as eb_pool,
            tc.tile_pool(name="pexp", bufs=6) as pexp_pool,
            tc.tile_pool(name="pmul", bufs=4) as pmul_pool,
            tc.tile_pool(name="rec", bufs=2) as rec_pool,
            tc.tile_pool(name="psl", bufs=3, space="PSUM") as psl_pool,
            tc.tile_pool(name="psy", bufs=1, space="PSUM") as psy_pool,
            tc.tile_pool(name="psp", bufs=1, space="PSUM") as psp_pool,
        ):
            eb_t = {}
            for h in range(HPC):
                o0, o1 = _CHUNK_OFF[0], _CHUNK_OFF[1]
                eb_t[(h, 0)] = eb_pool.tile([P, WMAX], bf16, name="ebt")
                eng(h).dma_start(
                    eb_t[(h, 0)][:, 0 : o1 - o0], ebias[ds(h * P, P), o0:o1]
                )
                eng(h).dma_start(va_t[h][:, 0:QC], vat[ds(h * P, P), 0:QC])
            for h in range(HPC):
                e = eng(h)
                e.dma_start(qk_t[h][:, QC:T], qkt[ds(h * D, D), QC:T])
                e.dma_start(qk_t[h][:, T + QC :], qkt[ds(h * D, D), T + QC :])
                e.dma_start(va_t[h][:, QC:], vat[ds(h * P, P), QC:])
            nc.gpsimd.dma_start(w2[:], wproj[:])

            for j in range(NJ):
                ntj = 4 * (j + 1)
                # prefetch next chunk's ebias (bufs=3 throttles lookahead)
                if j + 1 < NJ:
                    o0, o1 = _CHUNK_OFF[j + 1], _CHUNK_OFF[j + 2]
                    for h in range(HPC):
                        tl = eb_pool.tile([P, WMAX], bf16, name="ebt")
                        eb_t[(h, j + 1)] = tl
                        mid = (o1 - o0) // 2
                        eng(h).dma_start(
                            tl[:, 0:mid], ebias[ds(h * P, P), o0 : o0 + mid]
                        )
                        eng(h).dma_start(
                            tl[:, mid : o1 - o0],
                            ebias[ds(h * P, P), o0 + mid : o1],
                        )
                for h in range(HPC):
                    qT = qk_t[h][:, 0:T]
                    kT = qk_t[h][:, T : 2 * T]
                    ebh = eb_t[(h, j)]
                    psy_t = psy_pool.tile([P, QC], f32)
                    for g in range(ntj // G):
                        tiles = list(range(g * G, (g + 1) * G))
                        # packed (trimmed) in-tile offsets for BOTH the
                        # PSUM logits and the exp output: the whole group
                        # needs a single exp and a single DVE mult
                        loff = {}
                        acc = 0
                        for t, i in enumerate(tiles):
                            loff[t] = acc
                            acc += _W[(j, i)]
                        psl_t = psl_pool.tile([P, G * QC], f32)
                        for t, i in enumerate(tiles):
                            c0, w = _C0[(j, i)], _W[(j, i)]
                            nc.tensor.matmul(
                                psl_t[:, loff[t] : loff[t] + w],
                                lhsT=kT[:, i * P : (i + 1) * P],
                                rhs=qT[:, j * QC + c0 : (j + 1) * QC],
                                start=True,
                                stop=True,
                            )
                        pe_t = pexp_pool.tile([P, G * QC], bf16)
                        full = all(_C0[(j, i)] == 0 for i in tiles)
                        if full:
                            nc.scalar.activation(pe_t[:], psl_t[:], EXP)
                        elif j > 0:
                            # warm pipeline: one exp over the packed group
                            nc.scalar.activation(
                                pe_t[:, 0:acc], psl_t[:, 0:acc], EXP
                            )
                        else:
                            for t, i in enumerate(tiles):
                                w = _W[(j, i)]
                                nc.scalar.activation(
                                    pe_t[:, loff[t] : loff[t] + w],
                                    psl_t[:, loff[t] : loff[t] + w],
                                    EXP,
                                )
                        pm_t = pmul_pool.tile([P, G * QC], bf16)
                        b0 = _BOFF[(j, tiles[0])] - _CHUNK_OFF[j]
                        nc.vector.tensor_mul(
                            pm_t[:, 0:acc], pe_t[:, 0:acc],
                            ebh[:, b0 : b0 + acc],
                        )
                        for t, i in enumerate(tiles):
                            c0, w = _C0[(j, i)], _W[(j, i)]
                            nc.tensor.matmul(
                                psy_t[:, c0:QC],
                                lhsT=va_t[h][:, i * P : (i + 1) * P],
                                rhs=pm_t[:, loff[t] : loff[t] + w],
                                start=(i == 0),
                                stop=(i == ntj - 1),
                            )
                    # rows 64:128 of psy hold the softmax denominators
                    rec_t = rec_pool.tile([D, QC], f32)
                    if h == 0:
                        ydst = yT2[0:D, ds(j * QC, QC)]
                    elif h == 1:
                        ydst = yT2[D : 2 * D, ds(j * QC, QC)]
                    else:
                        ydst = yTs[:, ds(j * QC, QC)]
                    if j == NJ - 1 and h == HPC - 1:
                        # final head: halves, so the tail projection's first
                        # blocks start as soon as their columns are normed
                        for q0 in (0, QC // 2):
                            qs = slice(q0, q0 + QC // 2)
                            nc.vector.reciprocal(
                                rec_t[:, qs], psy_t[D : 2 * D, qs]
                            )
                            nc.vector.tensor_mul(
                                ydst[:, qs], psy_t[0:D, qs], rec_t[:, qs]
                            )
                    else:
                        nc.vector.reciprocal(rec_t[:], psy_t[D : 2 * D, :])
                        nc.vector.tensor_mul(ydst, psy_t[0:D, :], rec_t[:])

                    # interleave projection of the previous chunk: one or
                    # two tb-blocks after each head's attention
                    if j > 0:
                        tbs = ([4 * (j - 1)], [4 * (j - 1) + 1],
                               [4 * (j - 1) + 2, 4 * (j - 1) + 3])[h]
                        for tb in tbs:
                            _proj_tb(nc, tb, yT2, yTs, w2, o_big,
                                     psp_pool, f32)
                        if h == HPC - 1:
                            _out_dma(nc, j - 1, o_big, out)
            # tail projection: by now the attention pipeline has drained,
            # so borrow the 3-slot psl pool (same tag/shape) for 3-deep
            # PSUM pipelining and copy each block in one wide ACT instr
            # (ACT is idle once the last exp retires)
            for tb in range(4 * (NJ - 1), 4 * NJ):
                pst = psl_pool.tile([P, G * QC], f32, name="psl_t")
                for o0, ow in ((0, 512), (512, 256)):
                    nc.tensor.matmul(
                        pst[:, o0 : o0 + ow],
                        lhsT=yT2[:, tb * P : (tb + 1) * P],
                        rhs=w2[:, o0 : o0 + ow],
                        start=True,
                        stop=False,
                    )
                    nc.tensor.matmul(
                        pst[:, o0 : o0 + ow],
                        lhsT=yTs[:, tb * P : (tb + 1) * P],
                        rhs=w2[0:D, DIM + o0 : DIM + o0 + ow],
                        start=False,
                        stop=True,
                    )
                nc.scalar.copy(
                    o_big[:, tb * DIM : (tb + 1) * DIM], pst[:, 0:DIM]
                )
                eng3 = nc.gpsimd if tb % 2 == 0 else nc.sync
                eng3.dma_start(
                    out[tb * P : (tb + 1) * P, :].rearrange(
                        "(a p) o -> p a o", p=P
                    ),
                    o_big[:, tb * DIM : (tb + 1) * DIM].rearrange(
                        "p (a o) -> p a o", a=1
                    ),
                )

    nc.finalize()
    return nc


def _proj_tb(nc, tb, yT2, yTs, w2, o_big, psp_pool, f32):
    """Projection for one 128-row output block tb.  Late blocks copy
    PSUM->SBUF on ACT (idle once the exps are done); earlier ones on DVE."""
    for o0, ow in ((0, 512), (512, 256)):
        psp_t = psp_pool.tile([P, ow], f32, name="psp_t")
        nc.tensor.matmul(
            psp_t[:],
            lhsT=yT2[:, tb * P : (tb + 1) * P],
            rhs=w2[:, o0 : o0 + ow],
            start=True,
            stop=False,
        )
        nc.tensor.matmul(
            psp_t[:],
            lhsT=yTs[:, tb * P : (tb + 1) * P],
            rhs=w2[0:D, DIM + o0 : DIM + o0 + ow],
            start=False,
            stop=True,
        )
        if tb >= 12:
            nc.scalar.copy(
                o_big[:, tb * DIM + o0 : tb * DIM + o0 + ow], psp_t[:]
            )
        else:
            nc.vector.tensor_copy(
                o_big[:, tb * DIM + o0 : tb * DIM + o0 + ow], psp_t[:]
            )


def _out_dma(nc, c, o_big, out, nblk=4):
    """DMA output blocks [c*nblk, (c+1)*nblk) to HBM (c in 4-block units
    when nblk=4; in 2-block units offset by 6 when nblk=2)."""
    b0 = c * 4 if nblk == 4 else (c - 12) * 2 + 12
    eng = nc.sync if c % 2 == 0 else nc.gpsimd
    eng.dma_start(
        out[b0 * P : (b0 + nblk) * P, :].rearrange("(a p) o -> p a o", p=P),
        o_big[:, b0 * DIM : (b0 + nblk) * DIM].rearrange(
            "p (a o) -> p a o", a=nblk
        ),
    )


def _get_program():
    global _PROGRAM
    if _PROGRAM is None:
        _PROGRAM = _build_program()
    return _PROGRAM


def make_in_maps(q, k, v, attn_bias, W_proj):
    """Host-side sharding/layout prep: one input map per core."""
    q = np.asarray(q, dtype=np.float32)
    k = np.asarray(k, dtype=np.float32)
    v = np.asarray(v, dtype=np.float32)
    attn_bias = np.asarray(attn_bias, dtype=np.float32)
    W_proj = np.asarray(W_proj, dtype=np.float32)

    scale = 1.0 / math.sqrt(D)
    # causal mask in transposed [s, q] coords: masked (zeroed) where s > q
    smask = np.arange(T)[:, None] > np.arange(T)[None, :]
    w_heads = W_proj.reshape(H, D, DIM)

    in_maps = []
    for c in range(NCORES):
        b = c // 4
        h0 = HPC * (c % 4)
        hs = slice(h0, h0 + HPC)

        # vat: per head [128, T]: 16 blocks of [v-tile(128x64) | ones]
        va = np.empty((HPC, P, NT, P), dtype=np.float32)
        va[:, :, :, :D] = v[b, hs].reshape(HPC, NT, P, D).transpose(0, 2, 1, 3)
        va[:, :, :, D:] = 1.0

        # qkt: per head [64, 2T] = qT*scale | kT
        qk = np.empty((HPC, D, 2 * T), dtype=np.float32)
        qk[:, :, 0:T] = q[b, hs].transpose(0, 2, 1) * scale
        qk[:, :, T:] = k[b, hs].transpose(0, 2, 1)

        # ebias: exp(bias^T) with exact causal zeros, trimmed pack
        ebias_full = np.exp(attn_bias[b, hs].transpose(0, 2, 1))
        ebias_full[:, smask] = 0.0
        eb = np.empty((HPC, P, SUMW), dtype=np.float32)
        for j in range(NJ):
            for i in range(4 * (j + 1)):
                c0, b0 = _C0[(j, i)], _BOFF[(j, i)]
                eb[:, :, b0 : b0 + QC - c0] = ebias_full[
                    :, i * P : (i + 1) * P, j * QC + c0 : (j + 1) * QC
                ]

        w2 = np.zeros((P, 2 * DIM), dtype=np.float32)
        w2[0:D, 0:DIM] = w_heads[h0]
        w2[D:P, 0:DIM] = w_heads[h0 + 1]
        w2[0:D, DIM:] = w_heads[h0 + 2]

        in_maps.append(
            {
                "vat": va.reshape(HPC * P, T).astype(ml_dtypes.bfloat16),
                "qkt": qk.reshape(HPC * D, 2 * T).astype(ml_dtypes.bfloat16),
                "ebias": eb.reshape(HPC * P, SUMW).astype(ml_dtypes.bfloat16),
                "wproj": w2.astype(ml_dtypes.bfloat16),
            }
        )
    return in_maps


def assemble_output(results):
    """Sum the 4 per-core partial projections for each batch."""
    out = np.zeros((B, T, DIM), dtype=np.float32)
    for c in range(NCORES):
        out[c // 4] += np.asarray(results[c]["out"], dtype=np.float32)
    return out


def kernel(q, k, v, attn_bias, W_proj):
    from concourse.bass_utils import run_bass_kernel_spmd

    nc = _get_program()
    in_maps = make_in_maps(q, k, v, attn_bias, W_proj)
    res = run_bass_kernel_spmd(nc, in_maps, list(range(NCORES)))
    return assemble_output(res.results)
